# revision 20
# baseline (speedup 1.0000x reference)
"""Paged decode attention (GQA) on 8 trn2 NeuronCores.

Strategy (data parallel over sequences):
  - Host bin-packs the 32 sequences onto 8 cores (4 slots/core, LPT on valid
    block count) and builds, per core, a flat list of 4-token "chunks" to
    gather (only valid blocks -> ~2x traffic saving vs dense).
  - KV cache converted to bf16 host-side (halves HBM traffic; fp8 was tested
    and rejected: attention concentrates enough that e4m3's ~6% per-element
    error does not average out -> rel err 4-6e-2 > the 2e-2 tolerance).
  - The new-token K/V (k_new/v_new) is handled host-side: the 4-token chunk
    containing position len-1 is redirected to a small per-sequence "patch"
    row appended to the cache, so the device never scatters into the cache.
  - K is gathered with dma_gather(transpose=True): each 4-token chunk
    [4*KVH, DH] lands as [DH(partitions), 4*KVH, chunk] -- already transposed
    for the scores matmul, eliminating all K PE-transposes. Gathers are
    batched 256 indices (2 compute slices) per call to amortize SWDGE
    descriptor-generation on the Pool engine.
  - Scores for all 8 kv heads accumulate into one PSUM tile [128 rows, 512]
    (row = kvh*16 + slot*4 + g) using 8 matmuls with head-masked q
    stationaries (zeros elsewhere). The additive mask is applied by a 9th
    matmul (identity x mask) opening the accumulation group, so no DVE hop
    sits between scores and exp.
  - exp on ACT writes bf16 probs with fused row-sum accumulation.
  - probs transposed per 128-token group (4 PE transposes, bf16) -> PV
    matmuls (2 per group, 4x head-pair redundancy keeps instr count low)
    accumulate in PSUM across all iterations.
  - Device returns raw PV accumulators + row sums; host normalizes
    (exp-without-max softmax is exactly normalizable after the fact).
  - A handful of dummy matmuls during the startup dead-time warm the PE HAM
    clock-gate to 2.4 GHz before real work arrives.
"""

import numpy as np

B = 32
H = 32
KVH = 8
G = 4
DH = 128
BS = 16
NBLK = 128
NUM_BLOCKS = B * NBLK
SCALE = DH ** -0.5

NCORES = 8
SLOTS = 4           # sequences per core
CHUNK = 4           # tokens per gathered row
ROWF = KVH * DH     # 1024 floats per token
ELEM = CHUNK * ROWF  # 4096 elements per chunk row
NCH_CACHE = NUM_BLOCKS * BS // CHUNK   # 16384 chunks in the cache
GPB = BS // CHUNK   # chunk groups per block = 4
NEG = -1.0e30
N_WARM = 6          # dummy matmuls to warm the PE clock gate


def _bf16():
    import ml_dtypes
    return np.dtype(ml_dtypes.bfloat16)


def _schedule(lens):
    """LPT bin-packing of sequences onto cores, 4 slots each."""
    nch = [(l + CHUNK - 1) // CHUNK for l in lens]
    order = sorted(range(B), key=lambda s: -nch[s])
    loads = [0] * NCORES
    counts = [0] * NCORES
    assign = [[] for _ in range(NCORES)]
    for s in order:
        c = min(
            (c for c in range(NCORES) if counts[c] < SLOTS),
            key=lambda c: loads[c],
        )
        assign[c].append(s)
        loads[c] += nch[s]
        counts[c] += 1
    t_iter = max(1, max((l + 127) // 128 for l in loads))
    if t_iter % 2:
        t_iter += 1  # gathers are batched 2 slices per call
    return assign, nch, t_iter


def _host_prepare(q, k_new, v_new, k_cache, v_cache, block_tables, context_lens):
    bf16 = _bf16()
    lens = [int(x) for x in context_lens]
    bt = np.asarray(block_tables)
    assign, nch, T = _schedule(lens)

    kc_flat = np.ascontiguousarray(k_cache).reshape(NUM_BLOCKS * BS, ROWF)
    vc_flat = np.ascontiguousarray(v_cache).reshape(NUM_BLOCKS * BS, ROWF)
    kn = np.ascontiguousarray(k_new).reshape(B, ROWF)
    vn = np.ascontiguousarray(v_new).reshape(B, ROWF)

    # patch rows: the 4-token group holding position len-1, with that token's
    # row replaced by k_new/v_new
    kpatch = np.zeros((B, ELEM), np.float32)
    vpatch = np.zeros((B, ELEM), np.float32)
    for s in range(B):
        l = lens[s]
        g = (l - 1) // CHUNK
        blk = int(bt[s, g // GPB])
        base_slot = blk * BS + (g % GPB) * CHUNK
        krows = kc_flat[base_slot : base_slot + CHUNK].copy()
        vrows = vc_flat[base_slot : base_slot + CHUNK].copy()
        krows[(l - 1) % CHUNK] = kn[s]
        vrows[(l - 1) % CHUNK] = vn[s]
        kpatch[s] = krows.reshape(-1)
        vpatch[s] = vrows.reshape(-1)
    kc4 = np.concatenate(
        [kc_flat.reshape(NCH_CACHE, ELEM).astype(bf16), kpatch.astype(bf16)], axis=0
    )
    vc4 = np.concatenate(
        [vc_flat.reshape(NCH_CACHE, ELEM).astype(bf16), vpatch.astype(bf16)], axis=0
    )

    qs = np.asarray(q, np.float32) * SCALE
    per_core = []
    for c in range(NCORES):
        seqs = assign[c]
        n = T * 128
        cid = np.zeros(n, np.int64)          # chunk ids
        cslot = np.full(n, -1, np.int64)     # owning slot, -1 = padding
        cbase = np.zeros(n, np.int64)        # first token index of chunk
        clen = np.zeros(n, np.int64)         # owning seq len
        pos = 0
        for slot, s in enumerate(seqs):
            l = lens[s]
            ns = nch[s]
            gpatch = (l - 1) // CHUNK
            g = np.arange(ns)
            ids = bt[s, g // GPB].astype(np.int64) * GPB + g % GPB
            ids[gpatch] = NCH_CACHE + s
            cid[pos : pos + ns] = ids
            cslot[pos : pos + ns] = slot
            cbase[pos : pos + ns] = g * CHUNK
            clen[pos : pos + ns] = l
            pos += ns

        # gather index tensor [128, (T//2)*16] int16; one 256-index call per
        # 2 slices; index j of call ci lives at [j % 16, ci*16 + j//16],
        # replicated across the 8 16-partition groups
        idx = np.zeros((128, (T // 2) * 16), np.int16)
        for ci in range(T // 2):
            ids = cid[ci * 256 : (ci + 1) * 256]
            tile16 = ids.reshape(16, 16).T.astype(np.int16)   # [16, 16]
            idx[:, ci * 16 : (ci + 1) * 16] = np.tile(tile16, (8, 1))

        # additive mask [128 rows (k,s,g), T*512]; col (t, j, p) <-> token j
        # of the chunk at position t*128+p; identical for all kv heads
        row_slot = np.arange(16) // 4                             # [16]
        mask16 = np.full((16, T * 512), NEG, np.float32)
        for t in range(T):
            sl = cslot[t * 128 : (t + 1) * 128]                   # [128]
            tb = cbase[t * 128 : (t + 1) * 128]
            ln = clen[t * 128 : (t + 1) * 128]
            j = np.arange(CHUNK)[:, None]                         # [4,1]
            valid = (tb[None, :] + j < ln[None, :]) & (sl[None, :] >= 0)
            ok = (row_slot[:, None, None] == sl[None, None, :]) & valid[None]
            m = np.where(ok, 0.0, NEG).astype(np.float32)         # [16,4,128]
            mask16[:, t * 512 : (t + 1) * 512] = m.reshape(16, 512)
        mask = np.tile(mask16, (KVH, 1)).astype(bf16)             # [128, T*512]

        # head-masked q stationaries: qtm[:, k, k*16 + slot*4 + g] = q row
        qtm = np.zeros((128, KVH, 128), np.float32)
        for slot, s in enumerate(seqs):
            for k in range(KVH):
                for g in range(G):
                    row = k * 16 + slot * 4 + g
                    qtm[:, k, row] = qs[s, k * G + g, :]
        qtm = np.ascontiguousarray(qtm.reshape(128, KVH * 128)).astype(bf16)

        per_core.append(dict(idx=idx, mask=mask, qtm=qtm, seqs=seqs))
    return kc4, vc4, per_core, T, assign


# ---------------------------------------------------------------------------
# device program
# ---------------------------------------------------------------------------

def _build_program(T):
    import concourse.bass as bass  # noqa: F401
    import concourse.mybir as mybir
    import concourse.tile as tile
    from concourse import bacc
    from concourse.masks import make_identity

    f32 = mybir.dt.float32
    bf16 = mybir.dt.bfloat16
    i16 = mybir.dt.int16
    Act = mybir.ActivationFunctionType

    assert T % 2 == 0
    NCALLS = T // 2

    nc = bacc.Bacc(
        "TRN2", target_bir_lowering=False, debug=False, num_devices=NCORES
    )
    kc_d = nc.dram_tensor("kc4", [NCH_CACHE + B, ELEM], bf16, kind="ExternalInput")
    vc_d = nc.dram_tensor("vc4", [NCH_CACHE + B, ELEM], bf16, kind="ExternalInput")
    qtm_d = nc.dram_tensor("qtm", [128, KVH * 128], bf16, kind="ExternalInput")
    idx_d = nc.dram_tensor("idx", [128, NCALLS * 16], i16, kind="ExternalInput")
    mask_d = nc.dram_tensor("mask", [128, T * 512], bf16, kind="ExternalInput")
    out_d = nc.dram_tensor("o", [64, 1024], f32, kind="ExternalOutput")
    sums_d = nc.dram_tensor("s", [128, T], f32, kind="ExternalOutput")

    with tile.TileContext(nc) as tc:
        with (
            tc.tile_pool(name="const", bufs=1) as constp,
            tc.tile_pool(name="kbuf", bufs=2) as kpool,
            tc.tile_pool(name="vbuf", bufs=2) as vpool,
            tc.tile_pool(name="msk", bufs=2) as mpool,
            tc.tile_pool(name="prb", bufs=2) as ppool,
            tc.tile_pool(name="prt", bufs=2) as ptpool,
            tc.tile_pool(name="scp", bufs=2, space="PSUM") as spsum,
            tc.tile_pool(name="ptp", bufs=2, space="PSUM") as ptpsum,
            tc.tile_pool(name="pvp", bufs=1, space="PSUM") as pvpool,
            tc.tile_pool(name="wrm", bufs=1, space="PSUM") as warmpool,
        ):
            # idx rides the Sync queue alone so its completion sem fires ASAP
            # (the gathers are gated on it); bulk consts go on other engines'
            # queues
            idxs = constp.tile([128, NCALLS * 16], i16)
            nc.sync.dma_start(idxs[:], idx_d.ap())
            qtm = constp.tile([128, KVH * 128], bf16)
            nc.scalar.dma_start(qtm[:], qtm_d.ap())
            ident = constp.tile([128, 128], bf16)
            make_identity(nc, ident[:])
            partials = constp.tile([128, T], f32)

            pv = [
                pvpool.tile([64, 512], f32, name=f"pv{g2}", tag=f"pv{g2}")
                for g2 in range(2)
            ]
            qtm3 = qtm[:].rearrange("p (k m) -> p k m", k=KVH)

            # warm the PE clock gate during startup (results never read)
            warm = warmpool.tile([128, 512], f32, tag="warm")
            for _ in range(N_WARM):
                nc.tensor.matmul(
                    warm[:], lhsT=ident[:], rhs=qtm[:, 0:512],
                    start=True, stop=True, skip_group_check=True,
                )

            def emit_pv(probs, v_call, j2, t):
                ptp = ptpsum.tile([128, 512], bf16, tag="ptp")
                for t4 in range(CHUNK):
                    nc.tensor.transpose(
                        ptp[:, t4 * 128 : (t4 + 1) * 128],
                        probs[:, t4 * 128 : (t4 + 1) * 128],
                        ident[:],
                    )
                pt_sb = ptpool.tile([128, 512], bf16)
                nc.vector.tensor_copy(pt_sb[:], ptp[:])
                for t4 in range(CHUNK):
                    for g2 in range(2):
                        nc.tensor.matmul(
                            pv[g2][:],
                            lhsT=pt_sb[:, t4 * 128 + g2 * 64 : t4 * 128 + g2 * 64 + 64],
                            rhs=v_call[:, j2, t4 * 1024 + g2 * 512 : t4 * 1024 + (g2 + 1) * 512],
                            start=(t == 0 and t4 == 0),
                            stop=(t == T - 1 and t4 == CHUNK - 1),
                            skip_group_check=True,
                        )

            pending = None
            for ci in range(NCALLS):
                k_call = kpool.tile([128, 32, 256], bf16)
                nc.gpsimd.dma_gather(
                    k_call[:],
                    kc_d.ap(),
                    idxs[:, ci * 16 : (ci + 1) * 16],
                    num_idxs=256,
                    num_idxs_reg=256,
                    elem_size=ELEM,
                    transpose=True,
                    single_packet=False,
                )
                v_call = vpool.tile([128, 2, ELEM], bf16)
                nc.gpsimd.dma_gather(
                    v_call[:],
                    vc_d.ap(),
                    idxs[:, ci * 16 : (ci + 1) * 16],
                    num_idxs=256,
                    num_idxs_reg=256,
                    elem_size=ELEM,
                    single_packet=False,
                )
                mask_call = mpool.tile([128, 1024], bf16)
                nc.scalar.dma_start(
                    mask_call[:], mask_d.ap()[:, ci * 1024 : (ci + 1) * 1024]
                )
                # [128, 32, 256] -> per head k, slice j2: [128, t4(4), 128]
                kt4 = k_call[:].rearrange("p (t4 kv) (j n) -> p kv j t4 n",
                                          kv=KVH, j=2)
                for j2 in range(2):
                    t = ci * 2 + j2
                    sp = spsum.tile([128, 512], f32, tag="sp")
                    nc.tensor.matmul(
                        sp[:], lhsT=ident[:],
                        rhs=mask_call[:, j2 * 512 : (j2 + 1) * 512],
                        start=True, stop=False,
                    )
                    for k in range(KVH):
                        nc.tensor.matmul(
                            sp[:],
                            lhsT=qtm3[:, k],
                            rhs=kt4[:, k, j2],
                            start=False,
                            stop=(k == KVH - 1),
                        )
                    probs = ppool.tile([128, 512], bf16)
                    nc.scalar.activation(
                        probs[:], sp[:], Act.Exp,
                        accum_out=partials[:, t : t + 1],
                    )
                    # software pipeline: PV of slice t-1 lands after scores of
                    # t, so the PE never stalls on the ACT of the same slice
                    if pending is not None:
                        emit_pv(*pending)
                    pending = (probs, v_call, j2, t)
            emit_pv(*pending)

            out_sb = constp.tile([64, 1024], f32)
            for g2 in range(2):
                nc.vector.tensor_copy(out_sb[:, g2 * 512 : (g2 + 1) * 512], pv[g2][:])
            nc.sync.dma_start(out_d.ap(), out_sb[:])
            nc.sync.dma_start(sums_d.ap(), partials[:])

    nc.compile()
    return nc


_prog_cache = {}


def _get_program(T):
    if T not in _prog_cache:
        _prog_cache[T] = _build_program(T)
    return _prog_cache[T]


def _assemble(res_list, per_core, T):
    out = np.zeros((B, 1, H, DH), np.float32)
    for c in range(NCORES):
        o = np.asarray(res_list[c]["o"], np.float32)        # [64, 1024]
        sums = np.asarray(res_list[c]["s"], np.float32).sum(axis=1)  # [128]
        for slot, s in enumerate(per_core[c]["seqs"]):
            for k in range(KVH):
                g2, kl = divmod(k, 4)
                for g in range(G):
                    r64 = kl * 16 + slot * 4 + g
                    r128 = k * 16 + slot * 4 + g
                    vec = o[r64, g2 * 512 + kl * 128 : g2 * 512 + (kl + 1) * 128]
                    out[s, 0, k * G + g] = vec / max(sums[r128], 1e-30)
    return out


def kernel(q, k_new, v_new, k_cache, v_cache, block_tables, context_lens,
           slot_mapping, _trace=False):
    from concourse.bass_utils import run_bass_kernel_spmd

    kc4, vc4, per_core, T, assign = _host_prepare(
        q, k_new, v_new, k_cache, v_cache, block_tables, context_lens
    )
    nc = _get_program(T)

    in_maps = []
    for c in range(NCORES):
        pc = per_core[c]
        in_maps.append(
            {
                "kc4": kc4,
                "vc4": vc4,
                "qtm": pc["qtm"],
                "idx": pc["idx"],
                "mask": pc["mask"],
            }
        )
    res = run_bass_kernel_spmd(
        nc, in_maps, core_ids=list(range(NCORES)), trace=_trace
    )

    out = _assemble(res.results, per_core, T)
    if _trace:
        kernel._last_results = res
    return out


# revision 30
# speedup vs baseline: 1.1235x; 1.1235x over previous
"""Paged decode attention (GQA) on 8 trn2 NeuronCores.

Strategy (data parallel over sequences):
  - Host bin-packs the 32 sequences onto 8 cores (4 slots/core, LPT on valid
    block count) and builds, per core, a flat list of 4-token "chunks" to
    gather (only valid blocks -> ~2x traffic saving vs dense).
  - KV cache converted to bf16 host-side (halves HBM traffic; fp8 was tested
    and rejected: attention concentrates enough that e4m3's ~6% per-element
    error does not average out -> rel err 4-6e-2 > the 2e-2 tolerance).
  - The new-token K/V (k_new/v_new) is handled host-side: the 4-token chunk
    containing position len-1 is redirected to a small per-sequence "patch"
    row appended to the cache, so the device never scatters into the cache.
  - K is gathered with dma_gather(transpose=True): each 4-token chunk
    [4*KVH, DH] lands as [DH(partitions), 4*KVH, chunk] -- already transposed
    for the scores matmul, eliminating all K PE-transposes. Gathers are
    batched 256 indices (2 compute slices) per call to amortize SWDGE
    descriptor-generation on the Pool engine.
  - Scores for all 8 kv heads accumulate into one PSUM tile [128 rows, 512]
    (row = kvh*16 + slot*4 + g) using 8 matmuls with head-masked q
    stationaries (zeros elsewhere). The additive mask is applied by a 9th
    matmul (identity x mask) opening the accumulation group, so no DVE hop
    sits between scores and exp.
  - exp on ACT writes bf16 probs with fused row-sum accumulation.
  - probs transposed per 128-token group (4 PE transposes, bf16) -> PV
    matmuls (2 per group, 4x head-pair redundancy keeps instr count low)
    accumulate in PSUM across all iterations.
  - Device returns raw PV accumulators + row sums; host normalizes
    (exp-without-max softmax is exactly normalizable after the fact).
  - A handful of dummy matmuls during the startup dead-time warm the PE HAM
    clock-gate to 2.4 GHz before real work arrives.
"""

import numpy as np

B = 32
H = 32
KVH = 8
G = 4
DH = 128
BS = 16
NBLK = 128
NUM_BLOCKS = B * NBLK
SCALE = DH ** -0.5

NCORES = 8
SLOTS = 4           # sequences per core
CHUNK = 4           # tokens per gathered row
ROWF = KVH * DH     # 1024 floats per token
ELEM = CHUNK * ROWF  # 4096 elements per chunk row
NCH_CACHE = NUM_BLOCKS * BS // CHUNK   # 16384 chunks in the cache
GPB = BS // CHUNK   # chunk groups per block = 4
NEG = -1.0e30
N_WARM = 6          # dummy matmuls to warm the PE clock gate


def _bf16():
    import ml_dtypes
    return np.dtype(ml_dtypes.bfloat16)


def _schedule(lens):
    """LPT bin-packing of sequences onto cores, 4 slots each."""
    nch = [(l + CHUNK - 1) // CHUNK for l in lens]
    order = sorted(range(B), key=lambda s: -nch[s])
    loads = [0] * NCORES
    counts = [0] * NCORES
    assign = [[] for _ in range(NCORES)]
    for s in order:
        c = min(
            (c for c in range(NCORES) if counts[c] < SLOTS),
            key=lambda c: loads[c],
        )
        assign[c].append(s)
        loads[c] += nch[s]
        counts[c] += 1
    t_iter = max(1, max((l + 127) // 128 for l in loads))
    if t_iter % 2:
        t_iter += 1  # gathers are batched 2 slices per call
    return assign, nch, t_iter


def _host_prepare(q, k_new, v_new, k_cache, v_cache, block_tables, context_lens):
    bf16 = _bf16()
    lens = [int(x) for x in context_lens]
    bt = np.asarray(block_tables)
    assign, nch, T = _schedule(lens)

    kc_flat = np.ascontiguousarray(k_cache).reshape(NUM_BLOCKS * BS, ROWF)
    vc_flat = np.ascontiguousarray(v_cache).reshape(NUM_BLOCKS * BS, ROWF)
    kn = np.ascontiguousarray(k_new).reshape(B, ROWF)
    vn = np.ascontiguousarray(v_new).reshape(B, ROWF)

    # patch rows: the 4-token group holding position len-1, with that token's
    # row replaced by k_new/v_new
    kpatch = np.zeros((B, ELEM), np.float32)
    vpatch = np.zeros((B, ELEM), np.float32)
    for s in range(B):
        l = lens[s]
        g = (l - 1) // CHUNK
        blk = int(bt[s, g // GPB])
        base_slot = blk * BS + (g % GPB) * CHUNK
        krows = kc_flat[base_slot : base_slot + CHUNK].copy()
        vrows = vc_flat[base_slot : base_slot + CHUNK].copy()
        krows[(l - 1) % CHUNK] = kn[s]
        vrows[(l - 1) % CHUNK] = vn[s]
        kpatch[s] = krows.reshape(-1)
        vpatch[s] = vrows.reshape(-1)
    kc4 = np.concatenate(
        [kc_flat.reshape(NCH_CACHE, ELEM).astype(bf16), kpatch.astype(bf16)], axis=0
    )
    vc4 = np.concatenate(
        [vc_flat.reshape(NCH_CACHE, ELEM).astype(bf16), vpatch.astype(bf16)], axis=0
    )

    qs = np.asarray(q, np.float32) * SCALE
    per_core = []
    for c in range(NCORES):
        seqs = assign[c]
        n = T * 128
        cid = np.zeros(n, np.int64)          # chunk ids
        cslot = np.full(n, -1, np.int64)     # owning slot, -1 = padding
        cbase = np.zeros(n, np.int64)        # first token index of chunk
        clen = np.zeros(n, np.int64)         # owning seq len
        pos = 0
        for slot, s in enumerate(seqs):
            l = lens[s]
            ns = nch[s]
            gpatch = (l - 1) // CHUNK
            g = np.arange(ns)
            ids = bt[s, g // GPB].astype(np.int64) * GPB + g % GPB
            ids[gpatch] = NCH_CACHE + s
            cid[pos : pos + ns] = ids
            cslot[pos : pos + ns] = slot
            cbase[pos : pos + ns] = g * CHUNK
            clen[pos : pos + ns] = l
            pos += ns

        # gather index tensor [128, (T//2)*16] int16; one 256-index call per
        # 2 slices; index j of call ci lives at [j % 16, ci*16 + j//16],
        # replicated across the 8 16-partition groups
        idx = np.zeros((128, (T // 2) * 16), np.int16)
        for ci in range(T // 2):
            ids = cid[ci * 256 : (ci + 1) * 256]
            tile16 = ids.reshape(16, 16).T.astype(np.int16)   # [16, 16]
            idx[:, ci * 16 : (ci + 1) * 16] = np.tile(tile16, (8, 1))

        # additive mask [128 rows (k,s,g), T*512]; col (t, j, p) <-> token j
        # of the chunk at position t*128+p; identical for all kv heads
        row_slot = np.arange(16) // 4                             # [16]
        mask16 = np.full((16, T * 512), NEG, np.float32)
        for t in range(T):
            sl = cslot[t * 128 : (t + 1) * 128]                   # [128]
            tb = cbase[t * 128 : (t + 1) * 128]
            ln = clen[t * 128 : (t + 1) * 128]
            j = np.arange(CHUNK)[:, None]                         # [4,1]
            valid = (tb[None, :] + j < ln[None, :]) & (sl[None, :] >= 0)
            ok = (row_slot[:, None, None] == sl[None, None, :]) & valid[None]
            m = np.where(ok, 0.0, NEG).astype(np.float32)         # [16,4,128]
            mask16[:, t * 512 : (t + 1) * 512] = m.reshape(16, 512)
        mask = np.tile(mask16, (KVH, 1)).astype(bf16)             # [128, T*512]

        # head-masked q stationaries: qtm[:, k, k*16 + slot*4 + g] = q row;
        # a 128x128 identity rides along as the last block (used as the
        # mask-matmul/transpose stationary -- keeps the Pool engine free for
        # the gather ucode library load)
        qtm = np.zeros((128, KVH + 1, 128), np.float32)
        for slot, s in enumerate(seqs):
            for k in range(KVH):
                for g in range(G):
                    row = k * 16 + slot * 4 + g
                    qtm[:, k, row] = qs[s, k * G + g, :]
        qtm[:, KVH, :] = np.eye(128, dtype=np.float32)
        qtm = np.ascontiguousarray(qtm.reshape(128, (KVH + 1) * 128)).astype(bf16)

        per_core.append(dict(idx=idx, mask=mask, qtm=qtm, seqs=seqs))
    return kc4, vc4, per_core, T, assign


# ---------------------------------------------------------------------------
# device program
# ---------------------------------------------------------------------------

def _build_program(T):
    import concourse.bass as bass  # noqa: F401
    import concourse.mybir as mybir
    import concourse.tile as tile
    from concourse import bacc, library_config

    f32 = mybir.dt.float32
    bf16 = mybir.dt.bfloat16
    i16 = mybir.dt.int16
    Act = mybir.ActivationFunctionType

    assert T % 2 == 0
    NCALLS = T // 2

    nc = bacc.Bacc(
        "TRN2", target_bir_lowering=False, debug=False, num_devices=NCORES
    )
    kc_d = nc.dram_tensor("kc4", [NCH_CACHE + B, ELEM], bf16, kind="ExternalInput")
    vc_d = nc.dram_tensor("vc4", [NCH_CACHE + B, ELEM], bf16, kind="ExternalInput")
    qtm_d = nc.dram_tensor("qtm", [128, (KVH + 1) * 128], bf16, kind="ExternalInput")
    idx_d = nc.dram_tensor("idx", [128, NCALLS * 16], i16, kind="ExternalInput")
    mask_d = nc.dram_tensor("mask", [128, T * 512], bf16, kind="ExternalInput")
    out_d = nc.dram_tensor("o", [64, 1024], f32, kind="ExternalOutput")
    sums_d = nc.dram_tensor("s", [128, T], f32, kind="ExternalOutput")

    with tile.TileContext(nc) as tc:
        with (
            tc.tile_pool(name="const", bufs=1) as constp,
            tc.tile_pool(name="kbuf", bufs=2) as kpool,
            tc.tile_pool(name="vbuf", bufs=2) as vpool,
            tc.tile_pool(name="msk", bufs=2) as mpool,
            tc.tile_pool(name="prb", bufs=2) as ppool,
            tc.tile_pool(name="prt", bufs=2) as ptpool,
            tc.tile_pool(name="scp", bufs=2, space="PSUM") as spsum,
            tc.tile_pool(name="ptp", bufs=2, space="PSUM") as ptpsum,
            tc.tile_pool(name="pvp", bufs=1, space="PSUM") as pvpool,
            tc.tile_pool(name="wrm", bufs=1, space="PSUM") as warmpool,
        ):
            # start the Q7 ucode library load (needed by dma_gather, takes
            # ~10us) immediately -- nothing else runs on the Pool engine
            # before the first gather
            nc.gpsimd.load_library(library_config.mlp)
            idxs = constp.tile([128, NCALLS * 16], i16)
            nc.sync.dma_start(idxs[:], idx_d.ap())
            qtm = constp.tile([128, (KVH + 1) * 128], bf16)
            nc.sync.dma_start(qtm[:], qtm_d.ap())
            ident = qtm[:].rearrange("p (k m) -> p k m", k=KVH + 1)[:, KVH]
            partials = constp.tile([128, T], f32)

            pv = [
                pvpool.tile([64, 512], f32, name=f"pv{g2}", tag=f"pv{g2}")
                for g2 in range(2)
            ]
            qtm3 = qtm[:].rearrange("p (k m) -> p k m", k=KVH + 1)

            # warm the PE clock gate during startup (results never read)
            warm = warmpool.tile([128, 512], f32, tag="warm")
            for _ in range(N_WARM):
                nc.tensor.matmul(
                    warm[:], lhsT=ident, rhs=qtm[:, 0:512],
                    start=True, stop=True, skip_group_check=True,
                )

            def emit_pv(probs, v_call, j2, t):
                ptp = ptpsum.tile([128, 512], bf16, tag="ptp")
                for t4 in range(CHUNK):
                    nc.tensor.transpose(
                        ptp[:, t4 * 128 : (t4 + 1) * 128],
                        probs[:, t4 * 128 : (t4 + 1) * 128],
                        ident,
                    )
                pt_sb = ptpool.tile([128, 512], bf16)
                nc.vector.tensor_copy(pt_sb[:], ptp[:])
                for t4 in range(CHUNK):
                    for g2 in range(2):
                        nc.tensor.matmul(
                            pv[g2][:],
                            lhsT=pt_sb[:, t4 * 128 + g2 * 64 : t4 * 128 + g2 * 64 + 64],
                            rhs=v_call[:, j2, t4 * 1024 + g2 * 512 : t4 * 1024 + (g2 + 1) * 512],
                            start=(t == 0 and t4 == 0),
                            stop=(t == T - 1 and t4 == CHUNK - 1),
                            skip_group_check=True,
                        )

            pending = None
            for ci in range(NCALLS):
                k_call = kpool.tile([128, 32, 256], bf16)
                nc.gpsimd.dma_gather(
                    k_call[:],
                    kc_d.ap(),
                    idxs[:, ci * 16 : (ci + 1) * 16],
                    num_idxs=256,
                    num_idxs_reg=256,
                    elem_size=ELEM,
                    transpose=True,
                    single_packet=False,
                )
                v_call = vpool.tile([128, 2, ELEM], bf16)
                nc.gpsimd.dma_gather(
                    v_call[:],
                    vc_d.ap(),
                    idxs[:, ci * 16 : (ci + 1) * 16],
                    num_idxs=256,
                    num_idxs_reg=256,
                    elem_size=ELEM,
                    single_packet=False,
                )
                mask_call = mpool.tile([128, 1024], bf16)
                nc.sync.dma_start(
                    mask_call[:], mask_d.ap()[:, ci * 1024 : (ci + 1) * 1024]
                )
                # [128, 32, 256] -> per head k, slice j2: [128, t4(4), 128]
                kt4 = k_call[:].rearrange("p (t4 kv) (j n) -> p kv j t4 n",
                                          kv=KVH, j=2)
                for j2 in range(2):
                    t = ci * 2 + j2
                    sp = spsum.tile([128, 512], f32, tag="sp")
                    nc.tensor.matmul(
                        sp[:], lhsT=ident,
                        rhs=mask_call[:, j2 * 512 : (j2 + 1) * 512],
                        start=True, stop=False,
                    )
                    for k in range(KVH):
                        nc.tensor.matmul(
                            sp[:],
                            lhsT=qtm3[:, k],
                            rhs=kt4[:, k, j2],
                            start=False,
                            stop=(k == KVH - 1),
                        )
                    probs = ppool.tile([128, 512], bf16)
                    nc.scalar.activation(
                        probs[:], sp[:], Act.Exp,
                        accum_out=partials[:, t : t + 1],
                    )
                    # software pipeline: PV of slice t-1 lands after scores of
                    # t, so the PE never stalls on the ACT of the same slice
                    if pending is not None:
                        emit_pv(*pending)
                    pending = (probs, v_call, j2, t)
            emit_pv(*pending)

            out_sb = constp.tile([64, 1024], f32)
            for g2 in range(2):
                nc.vector.tensor_copy(out_sb[:, g2 * 512 : (g2 + 1) * 512], pv[g2][:])
            nc.sync.dma_start(out_d.ap(), out_sb[:])
            nc.sync.dma_start(sums_d.ap(), partials[:])

    nc.compile()
    return nc


_prog_cache = {}


def _get_program(T):
    if T not in _prog_cache:
        _prog_cache[T] = _build_program(T)
    return _prog_cache[T]


def _assemble(res_list, per_core, T):
    out = np.zeros((B, 1, H, DH), np.float32)
    for c in range(NCORES):
        o = np.asarray(res_list[c]["o"], np.float32)        # [64, 1024]
        sums = np.asarray(res_list[c]["s"], np.float32).sum(axis=1)  # [128]
        for slot, s in enumerate(per_core[c]["seqs"]):
            for k in range(KVH):
                g2, kl = divmod(k, 4)
                for g in range(G):
                    r64 = kl * 16 + slot * 4 + g
                    r128 = k * 16 + slot * 4 + g
                    vec = o[r64, g2 * 512 + kl * 128 : g2 * 512 + (kl + 1) * 128]
                    out[s, 0, k * G + g] = vec / max(sums[r128], 1e-30)
    return out


def kernel(q, k_new, v_new, k_cache, v_cache, block_tables, context_lens,
           slot_mapping, _trace=False):
    from concourse.bass_utils import run_bass_kernel_spmd

    kc4, vc4, per_core, T, assign = _host_prepare(
        q, k_new, v_new, k_cache, v_cache, block_tables, context_lens
    )
    nc = _get_program(T)

    in_maps = []
    for c in range(NCORES):
        pc = per_core[c]
        in_maps.append(
            {
                "kc4": kc4,
                "vc4": vc4,
                "qtm": pc["qtm"],
                "idx": pc["idx"],
                "mask": pc["mask"],
            }
        )
    res = run_bass_kernel_spmd(
        nc, in_maps, core_ids=list(range(NCORES)), trace=_trace
    )

    out = _assemble(res.results, per_core, T)
    if _trace:
        kernel._last_results = res
    return out


# revision 34
# speedup vs baseline: 1.1564x; 1.0293x over previous
"""Paged decode attention (GQA) on 8 trn2 NeuronCores.

Strategy (data parallel over sequences):
  - Host bin-packs the 32 sequences onto 8 cores (4 slots/core, LPT on valid
    block count) and builds, per core, a flat list of 4-token "chunks" to
    gather (only valid blocks -> ~2x traffic saving vs dense).
  - KV cache converted to bf16 host-side (halves HBM traffic; fp8 was tested
    and rejected: attention concentrates enough that e4m3's ~6% per-element
    error does not average out -> rel err 4-6e-2 > the 2e-2 tolerance).
  - The new-token K/V (k_new/v_new) is handled host-side: the 4-token chunk
    containing position len-1 is redirected to a small per-sequence "patch"
    row appended to the cache, so the device never scatters into the cache.
  - K is gathered with dma_gather(transpose=True): each 4-token chunk
    [4*KVH, DH] lands as [DH(partitions), 4*KVH, chunk] -- already transposed
    for the scores matmul, eliminating all K PE-transposes. Gathers are
    batched 256 indices (2 compute slices) per call to amortize SWDGE
    descriptor-generation on the Pool engine.
  - Scores for all 8 kv heads accumulate into one PSUM tile [128 rows, 512]
    (row = kvh*16 + slot*4 + g) using 8 matmuls with head-masked q
    stationaries (zeros elsewhere). The additive mask is applied by a 9th
    matmul (identity x mask) opening the accumulation group, so no DVE hop
    sits between scores and exp.
  - exp on ACT writes bf16 probs with fused row-sum accumulation.
  - probs transposed per 128-token group (4 PE transposes, bf16) -> PV
    matmuls (2 per group, 4x head-pair redundancy keeps instr count low)
    accumulate in PSUM across all iterations.
  - Device returns raw PV accumulators + row sums; host normalizes
    (exp-without-max softmax is exactly normalizable after the fact).
  - A handful of dummy matmuls during the startup dead-time warm the PE HAM
    clock-gate to 2.4 GHz before real work arrives.
"""

import numpy as np

B = 32
H = 32
KVH = 8
G = 4
DH = 128
BS = 16
NBLK = 128
NUM_BLOCKS = B * NBLK
SCALE = DH ** -0.5

NCORES = 8
SLOTS = 4           # sequences per core
CHUNK = 4           # tokens per gathered row
ROWF = KVH * DH     # 1024 floats per token
ELEM = CHUNK * ROWF  # 4096 elements per chunk row
NCH_CACHE = NUM_BLOCKS * BS // CHUNK   # 16384 chunks in the cache
GPB = BS // CHUNK   # chunk groups per block = 4
NEG = -1.0e30
N_WARM = 6          # dummy matmuls to warm the PE clock gate


def _bf16():
    import ml_dtypes
    return np.dtype(ml_dtypes.bfloat16)


def _schedule(lens):
    """LPT bin-packing of sequences onto cores, 4 slots each."""
    nch = [(l + CHUNK - 1) // CHUNK for l in lens]
    order = sorted(range(B), key=lambda s: -nch[s])
    loads = [0] * NCORES
    counts = [0] * NCORES
    assign = [[] for _ in range(NCORES)]
    for s in order:
        c = min(
            (c for c in range(NCORES) if counts[c] < SLOTS),
            key=lambda c: loads[c],
        )
        assign[c].append(s)
        loads[c] += nch[s]
        counts[c] += 1
    t_iter = max(1, max((l + 127) // 128 for l in loads))
    if t_iter % 2:
        t_iter += 1  # gathers are batched 2 slices per call
    return assign, nch, t_iter


def _host_prepare(q, k_new, v_new, k_cache, v_cache, block_tables, context_lens):
    bf16 = _bf16()
    lens = [int(x) for x in context_lens]
    bt = np.asarray(block_tables)
    assign, nch, T = _schedule(lens)

    kc_flat = np.ascontiguousarray(k_cache).reshape(NUM_BLOCKS * BS, ROWF)
    vc_flat = np.ascontiguousarray(v_cache).reshape(NUM_BLOCKS * BS, ROWF)
    kn = np.ascontiguousarray(k_new).reshape(B, ROWF)
    vn = np.ascontiguousarray(v_new).reshape(B, ROWF)

    # patch rows: the 4-token group holding position len-1, with that token's
    # row replaced by k_new/v_new
    kpatch = np.zeros((B, ELEM), np.float32)
    vpatch = np.zeros((B, ELEM), np.float32)
    for s in range(B):
        l = lens[s]
        g = (l - 1) // CHUNK
        blk = int(bt[s, g // GPB])
        base_slot = blk * BS + (g % GPB) * CHUNK
        krows = kc_flat[base_slot : base_slot + CHUNK].copy()
        vrows = vc_flat[base_slot : base_slot + CHUNK].copy()
        krows[(l - 1) % CHUNK] = kn[s]
        vrows[(l - 1) % CHUNK] = vn[s]
        kpatch[s] = krows.reshape(-1)
        vpatch[s] = vrows.reshape(-1)
    kc4 = np.concatenate(
        [kc_flat.reshape(NCH_CACHE, ELEM).astype(bf16), kpatch.astype(bf16)], axis=0
    )
    vc4 = np.concatenate(
        [vc_flat.reshape(NCH_CACHE, ELEM).astype(bf16), vpatch.astype(bf16)], axis=0
    )

    qs = np.asarray(q, np.float32) * SCALE
    per_core = []
    for c in range(NCORES):
        seqs = assign[c]
        n = T * 128
        cid = np.zeros(n, np.int64)          # chunk ids
        cslot = np.full(n, -1, np.int64)     # owning slot, -1 = padding
        cbase = np.zeros(n, np.int64)        # first token index of chunk
        clen = np.zeros(n, np.int64)         # owning seq len
        pos = 0
        for slot, s in enumerate(seqs):
            l = lens[s]
            ns = nch[s]
            gpatch = (l - 1) // CHUNK
            g = np.arange(ns)
            ids = bt[s, g // GPB].astype(np.int64) * GPB + g % GPB
            ids[gpatch] = NCH_CACHE + s
            cid[pos : pos + ns] = ids
            cslot[pos : pos + ns] = slot
            cbase[pos : pos + ns] = g * CHUNK
            clen[pos : pos + ns] = l
            pos += ns

        # gather index tensor [128, (T//2)*16] int16; one 256-index call per
        # 2 slices; index j of call ci lives at [j % 16, ci*16 + j//16],
        # replicated across the 8 16-partition groups
        idx = np.zeros((128, (T // 2) * 16), np.int16)
        for ci in range(T // 2):
            ids = cid[ci * 256 : (ci + 1) * 256]
            tile16 = ids.reshape(16, 16).T.astype(np.int16)   # [16, 16]
            idx[:, ci * 16 : (ci + 1) * 16] = np.tile(tile16, (8, 1))

        # additive mask [128 rows (k,s,g), T*512]; col (t, j, p) <-> token j
        # of the chunk at position t*128+p; identical for all kv heads
        row_slot = np.arange(16) // 4                             # [16]
        mask16 = np.full((16, T * 512), NEG, np.float32)
        for t in range(T):
            sl = cslot[t * 128 : (t + 1) * 128]                   # [128]
            tb = cbase[t * 128 : (t + 1) * 128]
            ln = clen[t * 128 : (t + 1) * 128]
            j = np.arange(CHUNK)[:, None]                         # [4,1]
            valid = (tb[None, :] + j < ln[None, :]) & (sl[None, :] >= 0)
            ok = (row_slot[:, None, None] == sl[None, None, :]) & valid[None]
            m = np.where(ok, 0.0, NEG).astype(np.float32)         # [16,4,128]
            mask16[:, t * 512 : (t + 1) * 512] = m.reshape(16, 512)
        mask = np.tile(mask16, (KVH, 1)).astype(bf16)             # [128, T*512]

        # head-masked q stationaries: qtm[:, k, k*16 + slot*4 + g] = q row;
        # a 128x128 identity rides along as the last block (used as the
        # mask-matmul/transpose stationary -- keeps the Pool engine free for
        # the gather ucode library load)
        qtm = np.zeros((128, KVH + 1, 128), np.float32)
        for slot, s in enumerate(seqs):
            for k in range(KVH):
                for g in range(G):
                    row = k * 16 + slot * 4 + g
                    qtm[:, k, row] = qs[s, k * G + g, :]
        qtm[:, KVH, :] = np.eye(128, dtype=np.float32)
        qtm = np.ascontiguousarray(qtm.reshape(128, (KVH + 1) * 128)).astype(bf16)

        # call 0's chunks prepacked dense (post-gather layout): plain HWDGE
        # DMAs fetch them during startup while the gather ucode library is
        # still loading on the Q7 cores
        ids0 = cid[0:256]
        k0 = np.ascontiguousarray(
            kc4[ids0].reshape(256, 32, 128).transpose(2, 1, 0)
        ).reshape(128, 32 * 256)
        v0 = np.ascontiguousarray(
            vc4[ids0].reshape(2, 128, ELEM).transpose(1, 0, 2)
        ).reshape(128, 2 * ELEM)

        per_core.append(dict(idx=idx, mask=mask, qtm=qtm, k0=k0, v0=v0,
                             seqs=seqs))
    return kc4, vc4, per_core, T, assign


# ---------------------------------------------------------------------------
# device program
# ---------------------------------------------------------------------------

def _build_program(T):
    import concourse.bass as bass  # noqa: F401
    import concourse.mybir as mybir
    import concourse.tile as tile
    from concourse import bacc, library_config

    f32 = mybir.dt.float32
    bf16 = mybir.dt.bfloat16
    i16 = mybir.dt.int16
    Act = mybir.ActivationFunctionType

    assert T % 2 == 0
    NCALLS = T // 2

    nc = bacc.Bacc(
        "TRN2", target_bir_lowering=False, debug=False, num_devices=NCORES
    )
    kc_d = nc.dram_tensor("kc4", [NCH_CACHE + B, ELEM], bf16, kind="ExternalInput")
    vc_d = nc.dram_tensor("vc4", [NCH_CACHE + B, ELEM], bf16, kind="ExternalInput")
    qtm_d = nc.dram_tensor("qtm", [128, (KVH + 1) * 128], bf16, kind="ExternalInput")
    idx_d = nc.dram_tensor("idx", [128, NCALLS * 16], i16, kind="ExternalInput")
    mask_d = nc.dram_tensor("mask", [128, T * 512], bf16, kind="ExternalInput")
    k0_d = nc.dram_tensor("k0", [128, 32 * 256], bf16, kind="ExternalInput")
    v0_d = nc.dram_tensor("v0", [128, 2 * ELEM], bf16, kind="ExternalInput")
    out_d = nc.dram_tensor("o", [64, 1024], f32, kind="ExternalOutput")
    sums_d = nc.dram_tensor("s", [128, T], f32, kind="ExternalOutput")

    with tile.TileContext(nc) as tc:
        with (
            tc.tile_pool(name="const", bufs=1) as constp,
            tc.tile_pool(name="kbuf", bufs=2) as kpool,
            tc.tile_pool(name="vbuf", bufs=2) as vpool,
            tc.tile_pool(name="msk", bufs=2) as mpool,
            tc.tile_pool(name="prb", bufs=2) as ppool,
            tc.tile_pool(name="prt", bufs=2) as ptpool,
            tc.tile_pool(name="scp", bufs=2, space="PSUM") as spsum,
            tc.tile_pool(name="ptp", bufs=2, space="PSUM") as ptpsum,
            tc.tile_pool(name="pvp", bufs=1, space="PSUM") as pvpool,
            tc.tile_pool(name="wrm", bufs=1, space="PSUM") as warmpool,
        ):
            # start the Q7 ucode library load (needed by dma_gather, takes
            # ~10us) immediately -- nothing else runs on the Pool engine
            # before the first gather
            nc.gpsimd.load_library(library_config.mlp)
            idxs = constp.tile([128, NCALLS * 16], i16)
            nc.sync.dma_start(idxs[:], idx_d.ap())
            qtm = constp.tile([128, (KVH + 1) * 128], bf16)
            nc.sync.dma_start(qtm[:], qtm_d.ap())
            ident = qtm[:].rearrange("p (k m) -> p k m", k=KVH + 1)[:, KVH]
            partials = constp.tile([128, T], f32)

            pv = [
                pvpool.tile([64, 512], f32, name=f"pv{g2}", tag=f"pv{g2}")
                for g2 in range(2)
            ]
            qtm3 = qtm[:].rearrange("p (k m) -> p k m", k=KVH + 1)

            # warm the PE clock gate during startup (results never read)
            warm = warmpool.tile([128, 512], f32, tag="warm")
            for _ in range(N_WARM):
                nc.tensor.matmul(
                    warm[:], lhsT=ident, rhs=qtm[:, 0:512],
                    start=True, stop=True, skip_group_check=True,
                )

            def emit_pv(probs, v_call, j2, t):
                ptp = ptpsum.tile([128, 512], bf16, tag="ptp")
                for t4 in range(CHUNK):
                    nc.tensor.transpose(
                        ptp[:, t4 * 128 : (t4 + 1) * 128],
                        probs[:, t4 * 128 : (t4 + 1) * 128],
                        ident,
                    )
                pt_sb = ptpool.tile([128, 512], bf16)
                nc.vector.tensor_copy(pt_sb[:], ptp[:])
                for t4 in range(CHUNK):
                    for g2 in range(2):
                        nc.tensor.matmul(
                            pv[g2][:],
                            lhsT=pt_sb[:, t4 * 128 + g2 * 64 : t4 * 128 + g2 * 64 + 64],
                            rhs=v_call[:, j2, t4 * 1024 + g2 * 512 : t4 * 1024 + (g2 + 1) * 512],
                            start=(t == 0 and t4 == 0),
                            stop=(t == T - 1 and t4 == CHUNK - 1),
                            skip_group_check=True,
                        )

            pending = None
            for ci in range(NCALLS):
                k_call = kpool.tile([128, 32, 256], bf16)
                v_call = vpool.tile([128, 2, ELEM], bf16)
                if ci == 0:
                    # prepacked dense: rides the Scalar HWDGE queue during
                    # the gather-library load
                    nc.scalar.dma_start(
                        k_call[:].rearrange("p c i -> p (c i)"), k0_d.ap()
                    )
                    nc.scalar.dma_start(
                        v_call[:].rearrange("p j f -> p (j f)"), v0_d.ap()
                    )
                else:
                    nc.gpsimd.dma_gather(
                        k_call[:],
                        kc_d.ap(),
                        idxs[:, ci * 16 : (ci + 1) * 16],
                        num_idxs=256,
                        num_idxs_reg=256,
                        elem_size=ELEM,
                        transpose=True,
                        single_packet=False,
                    )
                    nc.gpsimd.dma_gather(
                        v_call[:],
                        vc_d.ap(),
                        idxs[:, ci * 16 : (ci + 1) * 16],
                        num_idxs=256,
                        num_idxs_reg=256,
                        elem_size=ELEM,
                        single_packet=False,
                    )
                mask_call = mpool.tile([128, 1024], bf16)
                nc.sync.dma_start(
                    mask_call[:], mask_d.ap()[:, ci * 1024 : (ci + 1) * 1024]
                )
                # [128, 32, 256] -> per head k, slice j2: [128, t4(4), 128]
                kt4 = k_call[:].rearrange("p (t4 kv) (j n) -> p kv j t4 n",
                                          kv=KVH, j=2)
                for j2 in range(2):
                    t = ci * 2 + j2
                    sp = spsum.tile([128, 512], f32, tag="sp")
                    nc.tensor.matmul(
                        sp[:], lhsT=ident,
                        rhs=mask_call[:, j2 * 512 : (j2 + 1) * 512],
                        start=True, stop=False,
                    )
                    for k in range(KVH):
                        nc.tensor.matmul(
                            sp[:],
                            lhsT=qtm3[:, k],
                            rhs=kt4[:, k, j2],
                            start=False,
                            stop=(k == KVH - 1),
                        )
                    probs = ppool.tile([128, 512], bf16)
                    nc.scalar.activation(
                        probs[:], sp[:], Act.Exp,
                        accum_out=partials[:, t : t + 1],
                    )
                    # software pipeline: PV of slice t-1 lands after scores of
                    # t, so the PE never stalls on the ACT of the same slice
                    if pending is not None:
                        emit_pv(*pending)
                    pending = (probs, v_call, j2, t)
            emit_pv(*pending)

            out_sb = constp.tile([64, 1024], f32)
            for g2 in range(2):
                nc.vector.tensor_copy(out_sb[:, g2 * 512 : (g2 + 1) * 512], pv[g2][:])
            nc.sync.dma_start(out_d.ap(), out_sb[:])
            nc.sync.dma_start(sums_d.ap(), partials[:])

    nc.compile()
    return nc


_prog_cache = {}


def _get_program(T):
    if T not in _prog_cache:
        _prog_cache[T] = _build_program(T)
    return _prog_cache[T]


def _assemble(res_list, per_core, T):
    out = np.zeros((B, 1, H, DH), np.float32)
    for c in range(NCORES):
        o = np.asarray(res_list[c]["o"], np.float32)        # [64, 1024]
        sums = np.asarray(res_list[c]["s"], np.float32).sum(axis=1)  # [128]
        for slot, s in enumerate(per_core[c]["seqs"]):
            for k in range(KVH):
                g2, kl = divmod(k, 4)
                for g in range(G):
                    r64 = kl * 16 + slot * 4 + g
                    r128 = k * 16 + slot * 4 + g
                    vec = o[r64, g2 * 512 + kl * 128 : g2 * 512 + (kl + 1) * 128]
                    out[s, 0, k * G + g] = vec / max(sums[r128], 1e-30)
    return out


def kernel(q, k_new, v_new, k_cache, v_cache, block_tables, context_lens,
           slot_mapping, _trace=False):
    from concourse.bass_utils import run_bass_kernel_spmd

    kc4, vc4, per_core, T, assign = _host_prepare(
        q, k_new, v_new, k_cache, v_cache, block_tables, context_lens
    )
    nc = _get_program(T)

    in_maps = []
    for c in range(NCORES):
        pc = per_core[c]
        in_maps.append(
            {
                "kc4": kc4,
                "vc4": vc4,
                "qtm": pc["qtm"],
                "idx": pc["idx"],
                "mask": pc["mask"],
                "k0": pc["k0"],
                "v0": pc["v0"],
            }
        )
    res = run_bass_kernel_spmd(
        nc, in_maps, core_ids=list(range(NCORES)), trace=_trace
    )

    out = _assemble(res.results, per_core, T)
    if _trace:
        kernel._last_results = res
    return out
